# revision 2
# baseline (speedup 1.0000x reference)
"""Trainium2 Bass kernel v3 for LinearChainCrf NLL (B=256, T=1024, K=128), 8 cores.

Time-parallel exp-space CRF forward, 32 chunks of 32 steps:

  u_{s+1} = E'_{s+1} * (Wexp^T @ u_s),  E' = exp(e - beta) with beta = log K + 0.5.
  Host pre-transposes emissions to [K, t, B] bf16 with beta (and, for chunk 0,
  start_transitions) folded in, so the device does: DMA load -> ACT exp ->
  per-round {PE matmul, DVE tensor_mul}.

  Each core runs 2 groups x 2 chains: a group packs two 32-step chunks into one
  [K, 512] state tile (one PSUM bank per matmul), so each round is ONE matmul
  [128x128]@[128x512] and ONE DVE mul FD=512 per group. The two groups
  interleave so engines stay busy while each group's serial chain waits on
  semaphores. Warmup W=2 steps/chunk (CRF map contracts ~0.01/step; boundary
  stitching error ~1e-4 << tolerance).

  Captures: per group, PE colsum-matmuls (lhsT = [ones | exp(end)] [K,2]) of
  u_1 (A), u_31 (B for chunk 0), u_33 (B); all land in one PSUM bank per group
  at distinct partition rows, copied to SBUF at the end, one DMA out [12,512].
  Host stitches 32 chunk log-colsums telescopically (+1024*beta) into log Z and
  computes the gold path score; output nll = log_z - gold, [B] f32.
"""

from contextlib import ExitStack

import numpy as np

import concourse.bass as bass
from concourse import mybir
from concourse.bass_utils import run_bass_kernel_spmd

B, T, K = 256, 1024, 128
NCORES = 8
NCHUNK = 32          # total chunks
CHUNK = T // NCHUNK  # 32 steps per chunk
W = 1                # warmup steps per chunk (chunk 0: real steps)
S = CHUNK + W        # 33 rounds per chain
NG = 2               # groups per core
GC = 512             # batch-cols per group tile (2 chains x 256)
BL = [1, 1, 1, 1, 2, 2, 3, 4, 4, 4, 4, 4, 2]   # block sizes (load/exp grain)
assert sum(BL) == S
BSTART = [sum(BL[:i]) for i in range(len(BL))]
NBLK = len(BL)


def set_blocks(bl):
    """Dev hook: swap the load/exp block schedule (must sum to S)."""
    global BL, BSTART, NBLK
    assert sum(bl) == S
    BL = list(bl)
    BSTART = [sum(BL[:i]) for i in range(len(BL))]
    NBLK = len(BL)
BETA = float(np.log(K) + 0.5)
CAPS = [W - 1, CHUNK - 1, S - 1]    # rounds whose state u_s gets colsum-captured
FP32 = mybir.dt.float32
BF16 = mybir.dt.bfloat16
EXP = mybir.ActivationFunctionType.Exp

NB_NAT = 6
NB_ET = 5
NB_U = 2
NB_V = 2


def _blk_of(s):
    for b in range(NBLK):
        if s < BSTART[b] + BL[b]:
            return b, s - BSTART[b]
    raise ValueError(s)


def build_nc():
    nc = bass.Bass()
    em = nc.declare_dram_parameter("em", [K, S, NG * GC], BF16, isOutput=False)
    wexp = nc.declare_dram_parameter("wexp", [K, K], BF16, isOutput=False)
    colt = nc.declare_dram_parameter("colt", [K, 2], BF16, isOutput=False)
    # rows: 0 = ones-colsum, 1 = end-weighted; col block (k*NG + g)*GC
    out = nc.declare_dram_parameter("out", [2, NG * len(CAPS) * GC], FP32,
                                    isOutput=True)

    ctx = ExitStack()
    with ctx:
        sb = lambda name, shape, dt: ctx.enter_context(
            nc.sbuf_tensor(name, shape, dt))
        ps = lambda name, shape, dt: ctx.enter_context(
            nc.psum_tensor(name, shape, dt))

        wexp_sb = sb("wexp_sb", [K, K], BF16)
        colt_sb = sb("colt_sb", [K, 2], BF16)
        out_sb = sb("out_sb", [2, NG * len(CAPS) * GC], FP32)

        nat = [sb(f"nat{i}", [128, max(BL), NG * GC], BF16)
               for i in range(NB_NAT)]
        et = [sb(f"et{i}", [128, max(BL), NG * GC], BF16)
              for i in range(NB_ET)]
        u = [[sb(f"u{g}_{i}", [K, GC], BF16) for i in range(NB_U)]
             for g in range(NG)]

        v = [[ps(f"v{g}_{i}", [128, GC], FP32) for i in range(NB_V)]
             for g in range(NG)]
        # one bank per group for captures k=0,1 at partition rows 0/32
        # (matmul out base_partition must be 0/32/64); k=2 gets its own bank
        # so the final capture needn't wait for earlier copies to drain.
        cs = [ps(f"cs{g}", [34, GC], FP32) for g in range(NG)]
        cs2 = [ps(f"cs2_{g}", [2, GC], FP32) for g in range(NG)]

        sem_ctx = ExitStack()
        with sem_ctx:
            sm = lambda name: sem_ctx.enter_context(nc.semaphore(name))
            sW = sm("sW")                 # param loads
            sL = [sm(f"sL{i}") for i in range(NB_NAT)]
            sE = sm("sE")                 # exp blocks done
            sM = [sm(f"sM{g}") for g in range(NG)]   # PE instrs per group
            sT = [sm(f"sT{g}") for g in range(NG)]   # DVE muls per group
            sC = sm("sC")                 # ACT capture copies done (k=0,1)
            sO = sm("sO")                 # DVE final capture copies (k=2)
            sF = sm("sF")                 # out DMA done

            # PE instruction index bookkeeping per group:
            # round s in 1..S-1 -> matmul; capture after MM_{c+1} for c in CAPS
            # (except the last capture, issued after the final TT).
            mm_idx = [{} for _ in range(NG)]
            cap_idx = [{} for _ in range(NG)]
            for g in range(NG):
                n = 0
                for s in range(1, S):
                    n += 1
                    mm_idx[g][s] = n
                    if s - 1 in CAPS[:-1]:
                        n += 1
                        cap_idx[g][s - 1] = n
                n += 1
                cap_idx[g][CAPS[-1]] = n

            def et_slice(s, g):
                b, off = _blk_of(s)
                return et[b % NB_ET][:, off, g * GC:(g + 1) * GC]

            with nc.Block() as block:

                @block.scalar
                def _(act):
                    for b in range(NBLK):
                        act.wait_ge(sL[b % NB_NAT], 16 * (b // NB_NAT + 1))
                        if b >= NB_ET:
                            # et slot reuse: all muls of block b-NB_ET done
                            pb = b - NB_ET
                            last = BSTART[pb] + BL[pb] - 1
                            for g in range(NG):
                                act.wait_ge(sT[g], max(last, 1))
                                if pb == 0:
                                    # capture k=0 also reads et block 0
                                    act.wait_ge(sM[g], cap_idx[g][CAPS[0]])
                        nc.scalar.activation(
                            et[b % NB_ET][:, 0:BL[b], :],
                            nat[b % NB_NAT][:, 0:BL[b], :],
                            EXP,
                        ).then_inc(sE, 1)
                    # capture copies k=0,1: k-major so PE's later captures
                    # (same PSUM bank) can wait on sC thresholds
                    for k in range(2):
                        for g in range(NG):
                            act.wait_ge(sM[g], cap_idx[g][CAPS[k]])
                            cb = (k * NG + g) * GC
                            nc.scalar.copy(
                                out_sb[0:2, cb:cb + GC],
                                cs[g][32 * k:32 * k + 2, :],
                            ).then_inc(sC, 1)
                    # final capture copy for g=0 on ACT (g=1 goes to DVE)
                    k2 = len(CAPS) - 1
                    act.wait_ge(sM[0], cap_idx[0][CAPS[k2]])
                    cb = k2 * NG * GC
                    nc.scalar.copy(
                        out_sb[0:2, cb:cb + GC],
                        cs2[0][0:2, :],
                    ).then_inc(sO, 1)

                @block.tensor
                def _(pe):
                    pe.wait_ge(sW, 32)
                    pe.wait_ge(sE, 1)
                    # round 1: both groups' matmuls first, then the captures,
                    # so TT_1(g1) isn't serialized behind capture(g0)
                    for g in range(NG):
                        nc.tensor.matmul(
                            v[g][1 % NB_V][0:128, 0:GC], lhsT=wexp_sb[:, :],
                            rhs=et_slice(0, g), start=True, stop=True,
                        ).then_inc(sM[g], 1)
                    if 0 in CAPS[:-1]:
                        for g in range(NG):
                            nc.tensor.matmul(
                                cs[g][0:2, 0:GC], lhsT=colt_sb[:, :],
                                rhs=et_slice(0, g), start=True, stop=True,
                            ).then_inc(sM[g], 1)
                    for s in range(2, S):
                        for g in range(NG):
                            pe.wait_ge(sT[g], s - 1)
                            rhs = u[g][(s - 1) % NB_U][:, :]
                            nc.tensor.matmul(
                                v[g][s % NB_V][0:128, 0:GC], lhsT=wexp_sb[:, :],
                                rhs=rhs, start=True, stop=True,
                            ).then_inc(sM[g], 1)
                            if s - 1 in CAPS[:-1]:
                                k = CAPS.index(s - 1)
                                if k > 0:
                                    # cs bank reuse: prior captures copied out
                                    pe.wait_ge(sC, NG * k)
                                crhs = (et_slice(0, g) if s == 1
                                        else u[g][(s - 1) % NB_U][:, :])
                                nc.tensor.matmul(
                                    cs[g][32 * k:32 * k + 2, 0:GC],
                                    lhsT=colt_sb[:, :],
                                    rhs=crhs,
                                    start=True, stop=True,
                                ).then_inc(sM[g], 1)
                    for g in range(NG):
                        pe.wait_ge(sT[g], S - 1)
                        nc.tensor.matmul(
                            cs2[g][0:2, 0:GC], lhsT=colt_sb[:, :],
                            rhs=u[g][(S - 1) % NB_U][:, :],
                            start=True, stop=True,
                        ).then_inc(sM[g], 1)

                @block.vector
                def _(dv):
                    dv.wait_ge(sW, 32)
                    for s in range(1, S):
                        bb = _blk_of(s)[0]
                        for g in range(NG):
                            dv.wait_ge(sE, bb + 1)
                            dv.wait_ge(sM[g], mm_idx[g][s])
                            nc.vector.tensor_mul(
                                u[g][s % NB_U][:, :], v[g][s % NB_V][0:128, 0:GC],
                                et_slice(s, g)).then_inc(sT[g], 1)
                    # final capture copy for g=1 on DVE (idle after last round)
                    k = len(CAPS) - 1
                    dv.wait_ge(sM[1], cap_idx[1][CAPS[k]])
                    cb = (k * NG + 1) * GC
                    nc.vector.tensor_copy(
                        out_sb[0:2, cb:cb + GC],
                        cs2[1][0:2, :]).then_inc(sO, 1)

                @block.sync
                def _(sp):
                    def load(b):
                        sp.dma_start(
                            out=nat[b % NB_NAT][:, 0:BL[b], :],
                            in_=em[:, BSTART[b]:BSTART[b] + BL[b], :],
                        ).then_inc(sL[b % NB_NAT], 16)

                    load(0)
                    sp.dma_start(out=wexp_sb[:, :], in_=wexp[:, :]).then_inc(sW, 16)
                    sp.dma_start(out=colt_sb[:, :], in_=colt[:, :]).then_inc(sW, 16)
                    for b in range(1, min(NB_NAT, NBLK)):
                        load(b)
                    for b in range(NB_NAT, NBLK):
                        sp.wait_ge(sE, b - NB_NAT + 1)  # nat slot's exp drained
                        load(b)
                    # cols for k=0,1 go out early; k=2 cols in a small final DMA
                    sp.wait_ge(sC, NG * 2)
                    c2 = 2 * NG * GC
                    sp.dma_start(out=out[0:2, 0:c2],
                                 in_=out_sb[0:2, 0:c2]).then_inc(sF, 16)
                    sp.wait_ge(sO, 2)
                    sp.dma_start(out=out[0:2, c2:],
                                 in_=out_sb[0:2, c2:]).then_inc(sF, 16)
                    sp.wait_ge(sF, 32)
    return nc


_NC_CACHE = None


def get_nc():
    global _NC_CACHE
    if _NC_CACHE is None:
        _NC_CACHE = build_nc()
    return _NC_CACHE


def make_in_maps(emissions, transitions, start_transitions, end_transitions):
    import ml_dtypes
    bf16 = ml_dtypes.bfloat16
    y = (emissions - BETA).transpose(2, 1, 0).astype(bf16)   # [K, T, B]
    y[:, 0, :] += start_transitions.astype(bf16)[:, None]
    wexp = np.exp(transitions).astype(bf16)
    colt = np.ones((K, 2), np.float32)
    colt[:, 1] = np.exp(end_transitions)
    colt = colt.astype(bf16)

    in_maps = []
    for c in range(NCORES):
        chunks = [4 * c + j for j in range(4)]
        idx = np.empty((4, S), np.int64)
        for jj, j in enumerate(chunks):
            w0 = 0 if j == 0 else CHUNK * j - W
            idx[jj] = np.arange(w0, w0 + S)
        slab = y[:, idx, :]                      # [K, 4, S, B]
        slab = np.ascontiguousarray(slab.transpose(0, 2, 1, 3)).reshape(K, S, 4 * B)
        in_maps.append({"em": slab, "wexp": wexp, "colt": colt})
    return in_maps


def stitch(outs, tags, emissions, transitions, start_transitions,
           end_transitions):
    # outs[c]: [2, 3072] fp32; col block (k*NG+g)*512, rows (0 ones|1 end)
    capf = np.stack(outs)                        # [8, 2, 3072]
    logc = np.log(np.maximum(capf.astype(np.float64), 1e-300))

    def cap(j, k, row):
        c, r = divmod(j, 4)
        g, h = divmod(r, 2)
        cb = (k * NG + g) * GC + h * 256
        return logc[c, row, cb:cb + 256]

    # B_j: chunk 0 -> capture k=1 (u_31); j>=1 -> k=2 (u_33). A_j: k=0 (u_1).
    logz = cap(NCHUNK - 1, 2, 1).copy()          # end-weighted final colsum
    for j in range(1, NCHUNK):
        prev = cap(j - 1, 1, 0) if j == 1 else cap(j - 1, 2, 0)
        logz += prev - cap(j, 0, 0)
    logz += T * BETA

    tags_i = tags.astype(np.int64)
    gold = start_transitions[tags_i[:, 0]].astype(np.float64)
    gold = gold + end_transitions[tags_i[:, -1]]
    gold = gold + transitions[tags_i[:, :-1], tags_i[:, 1:]].sum(
        axis=1, dtype=np.float64)
    gold = gold + np.take_along_axis(
        emissions, tags_i[:, :, None], axis=2)[..., 0].sum(axis=1,
                                                           dtype=np.float64)
    return (logz - gold).astype(np.float32)


def kernel(emissions, transitions, start_transitions, end_transitions, tags, mask):
    emissions = np.asarray(emissions, dtype=np.float32)
    transitions = np.asarray(transitions, dtype=np.float32)
    start_transitions = np.asarray(start_transitions, dtype=np.float32)
    end_transitions = np.asarray(end_transitions, dtype=np.float32)
    tags = np.asarray(tags)
    assert np.asarray(mask).all(), "kernel assumes all-ones mask"

    in_maps = make_in_maps(emissions, transitions, start_transitions,
                           end_transitions)
    nc = get_nc()
    res = run_bass_kernel_spmd(nc, in_maps, core_ids=list(range(NCORES)))
    outs = [r["out"] for r in res.results]
    return stitch(outs, tags, emissions, transitions, start_transitions,
                  end_transitions)
